# revision 19
# baseline (speedup 1.0000x reference)
"""Masked single-query attention (N=64, T=2048, D=512) on 8 Trainium2 cores.

Reference computation per batch element n:
    energy[t] = sum_d key[t, n, :] . query[n, :]        (t < lens[n], else -1e9)
    attn      = softmax(energy)
    out[n]    = sum_t attn[t] * value[t, n, :]

Strategy:
  * Data-parallel over batch: each core handles 8 batch elements.
  * The mask means rows t >= lens[n] of key/value never contribute, so the
    host packs only the first lens[n] rows of K/V per element (rounded up to
    a 128-row chunk granularity) into per-core contiguous buffers, cutting
    HBM traffic roughly in half (memory-bound kernel).
  * All 8 cores must run the SAME program (SPMD), so the 64 batch elements
    are sorted by effective length and dealt round-robin into 8 "slots";
    slot j processes ceil(max_len_in_group / 128) chunks on every core.
    Rows past an element's own length are zero-padded; a per-(slot, chunk)
    additive mask vector (0 or -1e6) fed to the reduce's init-value operand
    reproduces the -1e9 masking exactly after softmax.
  * On-device per slot: energy via DVE tensor_tensor_reduce against the
    replicated query (keeps K in its natural [t, d] layout -- no transpose,
    and the 4x fp32 matmul penalty is avoided for the large contraction);
    softmax (max via PE transpose, exp+row-sum fused on ACT); context via
    PE matmuls attn_chunk.T @ V_chunk accumulated in PSUM.
  * lens[n] == 0 reproduces the reference exactly: softmax of an all-masked
    row is uniform over all T positions, so such elements are packed as
    2048 zero K rows (energy 0) with the full V.
"""

import sys

if "/opt/trn_rl_repo" not in sys.path:
    sys.path.insert(0, "/opt/trn_rl_repo")

import numpy as np

N, T, D = 64, 2048, 512
NCORES = 8
SLOTS_PER_CORE = N // NCORES
CHUNK = 128          # t-rows per energy/context chunk (partition dim)
SUB = 8              # chunks per DMA transfer ([128, SUB*512] tiles)
MASK_NEG = -1.0e6    # additive energy mask for padded rows

_PROGRAM_CACHE = {}


def _plan(lens):
    """Sort batch elements by effective length, deal into 8 slots x 8 cores.

    Returns (leff, cjs, assign) where assign[i][j] is the global batch index
    handled by core i in slot j, and cjs[j] is that slot's chunk count
    (shared by all cores so the SPMD program is uniform).
    """
    lens = np.asarray(lens).astype(np.int64)
    leff = np.where(lens == 0, T, lens)
    order = np.argsort(-leff, kind="stable")
    cjs = []
    assign = [[None] * SLOTS_PER_CORE for _ in range(NCORES)]
    for j in range(SLOTS_PER_CORE):
        grp = order[j * NCORES : (j + 1) * NCORES]
        cj = int(-(-int(leff[grp].max()) // CHUNK))  # ceil
        cjs.append(max(cj, 1))
        for i in range(NCORES):
            assign[i][j] = int(grp[i])
    return leff, tuple(cjs), assign


def _pack_inputs(query, key, value, leff, zero_lens, cjs, assign):
    """Build the per-core DRAM images: packed K/V, query row, energy mask.

    V is packed as bf16: the context contraction runs on the PE in bf16
    (fp32 matmuls cost 4 cycles/row vs 1 for bf16) and softmax weights are
    in [0, 1], so bf16 V costs ~2^-9 relative error while cutting the
    V half of HBM traffic in two (this kernel is HBM-bound).
    """
    import ml_dtypes

    query = np.ascontiguousarray(np.asarray(query, dtype=np.float32))
    key = np.asarray(key, dtype=np.float32)
    value = np.asarray(value, dtype=np.float32)
    totc = sum(cjs)
    in_maps = []
    for i in range(NCORES):
        # Partition-major packing [128, chunk, 512]: each SBUF partition's
        # DMA read is chunk-contiguous (8 KB runs for fp32 K at SUB=8)
        # instead of 2 KB strided rows -- bigger descriptors, better HBM rate.
        khat = np.zeros((CHUNK, totc, D), dtype=np.float32)
        vhat = np.zeros((CHUNK, totc, D), dtype=ml_dtypes.bfloat16)
        mask = np.zeros((CHUNK, totc), dtype=np.float32)
        qrow = np.zeros((1, SLOTS_PER_CORE * D), dtype=np.float32)
        col = 0
        for j, cj in enumerate(cjs):
            n = assign[i][j]
            L = int(leff[n])
            rows = cj * CHUNK
            # K rows: real rows for t < L unless the element is fully masked
            # (lens == 0 -> leff == T but energies must be 0, matching the
            # reference's uniform softmax over an all-masked row).
            kslot = np.zeros((rows, D), dtype=np.float32)
            if not zero_lens[n]:
                kslot[:L] = key[:L, n, :]
            khat[:, col : col + cj, :] = kslot.reshape(cj, CHUNK, D).transpose(
                1, 0, 2
            )
            vslot = np.zeros((rows, D), dtype=np.float32)
            vslot[:L] = value[:L, n, :]
            vhat[:, col : col + cj, :] = (
                vslot.astype(ml_dtypes.bfloat16)
                .reshape(cj, CHUNK, D)
                .transpose(1, 0, 2)
            )
            qrow[0, j * D : (j + 1) * D] = query[n]
            # mask: 0 where t < L else MASK_NEG, laid out [partition, chunk]
            t_idx = np.arange(rows).reshape(cj, CHUNK).T  # [128, cj]
            mask[:, col : col + cj] = np.where(t_idx < L, 0.0, MASK_NEG)
            col += cj
        in_maps.append(
            {"khat": khat, "vhat": vhat, "qrow": qrow, "maskv": mask}
        )
    return in_maps


def _build_program(cjs):
    """Trace the uniform SPMD Bass/Tile program for slot chunk counts cjs."""
    from contextlib import ExitStack

    import concourse.bass as bass
    import concourse.mybir as mybir
    from concourse import bacc, bass_isa, tile

    f32 = mybir.dt.float32
    bf16 = mybir.dt.bfloat16
    totc = sum(cjs)

    nc = bacc.Bacc("TRN2", target_bir_lowering=False, debug=False)
    kin = nc.dram_tensor("khat", [CHUNK, totc, D], f32, kind="ExternalInput").ap()
    vin = nc.dram_tensor("vhat", [CHUNK, totc, D], bf16, kind="ExternalInput").ap()
    qin = nc.dram_tensor(
        "qrow", [1, SLOTS_PER_CORE * D], f32, kind="ExternalInput"
    ).ap()
    min_ = nc.dram_tensor("maskv", [CHUNK, totc], f32, kind="ExternalInput").ap()
    out = nc.dram_tensor(
        "out", [SLOTS_PER_CORE, D], f32, kind="ExternalOutput"
    ).ap()

    with ExitStack() as ctx:
        tc = ctx.enter_context(tile.TileContext(nc))
        kpool = ctx.enter_context(tc.tile_pool(name="kpool", bufs=4))
        vpool = ctx.enter_context(tc.tile_pool(name="vpool", bufs=4))
        cpool = ctx.enter_context(tc.tile_pool(name="cpool", bufs=1))
        epool = ctx.enter_context(tc.tile_pool(name="epool", bufs=3))
        spool = ctx.enter_context(tc.tile_pool(name="spool", bufs=3))
        pcpool = ctx.enter_context(tc.tile_pool(name="pcpool", bufs=4, space="PSUM"))

        # ---- constants ----
        qsb = cpool.tile([1, SLOTS_PER_CORE * D], f32, tag="qsb")
        nc.scalar.dma_start(qsb[:], qin)
        masks = cpool.tile([CHUNK, totc], f32, tag="masks")
        nc.scalar.dma_start(masks[:], min_)
        dummy = cpool.tile([CHUNK, D], f32, tag="stt_dummy")

        # ---- replicate each slot's query to all 128 partitions ----
        qreps = []
        for j in range(SLOTS_PER_CORE):
            qr = cpool.tile([CHUNK, D], f32, tag=f"qrep{j}")
            nc.gpsimd.partition_broadcast(qr[:], qsb[0:1, j * D : (j + 1) * D])
            qreps.append(qr)

        col = 0
        for j, cj in enumerate(cjs):
            # ---------- energy phase ----------
            etile = epool.tile([CHUNK, cj], f32, tag="E")
            for s0 in range(0, cj, SUB):
                ns = min(SUB, cj - s0)
                ktile = kpool.tile([CHUNK, ns * D], f32, tag="kt")
                src = kin[:, col + s0 : col + s0 + ns, :]
                nc.sync.dma_start(ktile[:], src)
                for c in range(ns):
                    cs = s0 + c
                    nc.vector.scalar_tensor_tensor(
                        out=dummy[:],
                        in0=ktile[:, c * D : (c + 1) * D],
                        scalar=1.0,
                        in1=qreps[j][:],
                        op0=mybir.AluOpType.mult,
                        op1=mybir.AluOpType.mult,
                        accum_out=etile[:, cs : cs + 1],
                    )
            # apply the -1e6 padding mask
            nc.vector.tensor_add(etile[:], etile[:], masks[:, col : col + cj])

            # ---------- softmax ----------
            mx = spool.tile([CHUNK, 1], f32, tag="mx")
            nc.vector.reduce_max(mx[:], etile[:], axis=mybir.AxisListType.X)
            mall = spool.tile([CHUNK, 1], f32, tag="mall")
            nc.gpsimd.partition_all_reduce(
                mall[:], mx[:], CHUNK, bass_isa.ReduceOp.max
            )
            bias = spool.tile([CHUNK, 1], f32, tag="bias")
            nc.vector.tensor_scalar_mul(bias[:], mall[:], -1.0)
            atile = epool.tile([CHUNK, cj], bf16, tag="A")
            spart = spool.tile([CHUNK, 1], f32, tag="spart")
            nc.scalar.activation(
                atile[:],
                etile[:],
                mybir.ActivationFunctionType.Exp,
                bias=bias[:],
                scale=1.0,
                accum_out=spart[:],
            )
            sall = spool.tile([CHUNK, 1], f32, tag="sall")
            nc.gpsimd.partition_all_reduce(
                sall[:], spart[:], CHUNK, bass_isa.ReduceOp.add
            )
            rinv = spool.tile([1, 1], f32, tag="rinv")
            nc.vector.reciprocal(rinv[:], sall[0:1, 0:1])

            # ---------- context phase ----------
            pctx = pcpool.tile([1, D], f32, tag="pc")
            for s0 in range(0, cj, SUB):
                ns = min(SUB, cj - s0)
                vtile = vpool.tile([CHUNK, ns * D], bf16, tag="vt")
                src = vin[:, col + s0 : col + s0 + ns, :]
                nc.scalar.dma_start(vtile[:], src)
                for c in range(ns):
                    cs = s0 + c
                    nc.tensor.matmul(
                        pctx[:],
                        atile[:, cs : cs + 1],
                        vtile[:, c * D : (c + 1) * D],
                        start=(cs == 0),
                        stop=(cs == cj - 1),
                    )
            ob = spool.tile([1, D], f32, tag="ob")
            nc.scalar.mul(ob[:], pctx[:], rinv[:])
            nc.gpsimd.dma_start(out[j : j + 1, :], ob[:])

            col += cj

    nc.compile()
    return nc


def _get_program(cjs):
    if cjs not in _PROGRAM_CACHE:
        _PROGRAM_CACHE[cjs] = _build_program(cjs)
    return _PROGRAM_CACHE[cjs]


def run(query, key, value, lens, trace=False):
    """Run on 8 cores; returns (output [64, 512] fp32, BassKernelResults)."""
    from concourse.bass_utils import run_bass_kernel_spmd

    lens_arr = np.asarray(lens).astype(np.int64)
    zero_lens = lens_arr == 0
    leff, cjs, assign = _plan(lens_arr)
    nc = _get_program(cjs)
    in_maps = _pack_inputs(query, key, value, leff, zero_lens, cjs, assign)
    res = run_bass_kernel_spmd(
        nc, in_maps, list(range(NCORES)), trace=trace
    )
    out_full = np.empty((N, D), dtype=np.float32)
    for i in range(NCORES):
        ocore = res.results[i]["out"]
        for j in range(SLOTS_PER_CORE):
            out_full[assign[i][j]] = ocore[j]
    return out_full, res


def kernel(query, key, value, lens):
    out, _ = run(query, key, value, lens, trace=False)
    return out


# revision 21
# speedup vs baseline: 1.1912x; 1.1912x over previous
"""Masked single-query attention (N=64, T=2048, D=512) on 8 Trainium2 cores.

Reference computation per batch element n:
    energy[t] = sum_d key[t, n, :] . query[n, :]        (t < lens[n], else -1e9)
    attn      = softmax(energy)
    out[n]    = sum_t attn[t] * value[t, n, :]

Strategy:
  * Data-parallel over batch: each core handles 8 batch elements.
  * The mask means rows t >= lens[n] of key/value never contribute, so the
    host packs only the first lens[n] rows of K/V per element (rounded up to
    a 128-row chunk granularity) into per-core contiguous buffers, cutting
    HBM traffic roughly in half (memory-bound kernel).
  * All 8 cores must run the SAME program (SPMD), so the 64 batch elements
    are sorted by effective length and dealt round-robin into 8 "slots";
    slot j processes ceil(max_len_in_group / 128) chunks on every core.
    Rows past an element's own length are zero-padded; a per-(slot, chunk)
    additive mask vector (0 or -1e6) fed to the reduce's init-value operand
    reproduces the -1e9 masking exactly after softmax.
  * On-device per slot: energy via DVE tensor_tensor_reduce against the
    replicated query (keeps K in its natural [t, d] layout -- no transpose,
    and the 4x fp32 matmul penalty is avoided for the large contraction);
    softmax (max via PE transpose, exp+row-sum fused on ACT); context via
    PE matmuls attn_chunk.T @ V_chunk accumulated in PSUM.
  * lens[n] == 0 reproduces the reference exactly: softmax of an all-masked
    row is uniform over all T positions, so such elements are packed as
    2048 zero K rows (energy 0) with the full V.
"""

import sys

if "/opt/trn_rl_repo" not in sys.path:
    sys.path.insert(0, "/opt/trn_rl_repo")

import numpy as np

N, T, D = 64, 2048, 512
NCORES = 8
SLOTS_PER_CORE = N // NCORES
CHUNK = 128          # t-rows per energy/context chunk (partition dim)
SUB = 8              # chunks per DMA transfer ([128, SUB*512] tiles)
MASK_NEG = -1.0e6    # additive energy mask for padded rows

_PROGRAM_CACHE = {}


def _plan(lens):
    """Sort batch elements by effective length, deal into 8 slots x 8 cores.

    Returns (leff, cjs, assign) where assign[i][j] is the global batch index
    handled by core i in slot j, and cjs[j] is that slot's chunk count
    (shared by all cores so the SPMD program is uniform).
    """
    lens = np.asarray(lens).astype(np.int64)
    leff = np.where(lens == 0, T, lens)
    order = np.argsort(-leff, kind="stable")
    cjs = []
    assign = [[None] * SLOTS_PER_CORE for _ in range(NCORES)]
    for j in range(SLOTS_PER_CORE):
        grp = order[j * NCORES : (j + 1) * NCORES]
        cj = int(-(-int(leff[grp].max()) // CHUNK))  # ceil
        cjs.append(max(cj, 1))
        for i in range(NCORES):
            assign[i][j] = int(grp[i])
    return leff, tuple(cjs), assign


def _pack_inputs(query, key, value, leff, zero_lens, cjs, assign):
    """Build the per-core DRAM images: packed K/V, query row, energy mask.

    V is packed as bf16: the context contraction runs on the PE in bf16
    (fp32 matmuls cost 4 cycles/row vs 1 for bf16) and softmax weights are
    in [0, 1], so bf16 V costs ~2^-9 relative error while cutting the
    V half of HBM traffic in two (this kernel is HBM-bound).
    """
    import ml_dtypes

    query = np.ascontiguousarray(np.asarray(query, dtype=np.float32))
    key = np.asarray(key, dtype=np.float32)
    value = np.asarray(value, dtype=np.float32)
    totc = sum(cjs)
    in_maps = []
    for i in range(NCORES):
        # Partition-major packing [128, chunk, 512]: each SBUF partition's
        # DMA read is chunk-contiguous (8 KB runs for fp32 K at SUB=8)
        # instead of 2 KB strided rows -- bigger descriptors, better HBM rate.
        khat = np.zeros((CHUNK, totc, D), dtype=np.float32)
        vhat = np.zeros((CHUNK, totc, D), dtype=ml_dtypes.bfloat16)
        mask = np.zeros((CHUNK, totc), dtype=np.float32)
        qrow = np.zeros((1, SLOTS_PER_CORE * D), dtype=np.float32)
        col = 0
        for j, cj in enumerate(cjs):
            n = assign[i][j]
            L = int(leff[n])
            rows = cj * CHUNK
            # K rows: real rows for t < L unless the element is fully masked
            # (lens == 0 -> leff == T but energies must be 0, matching the
            # reference's uniform softmax over an all-masked row).
            kslot = np.zeros((rows, D), dtype=np.float32)
            if not zero_lens[n]:
                kslot[:L] = key[:L, n, :]
            khat[:, col : col + cj, :] = kslot.reshape(cj, CHUNK, D).transpose(
                1, 0, 2
            )
            vslot = np.zeros((rows, D), dtype=np.float32)
            vslot[:L] = value[:L, n, :]
            vhat[:, col : col + cj, :] = (
                vslot.astype(ml_dtypes.bfloat16)
                .reshape(cj, CHUNK, D)
                .transpose(1, 0, 2)
            )
            qrow[0, j * D : (j + 1) * D] = query[n]
            # mask: 0 where t < L else MASK_NEG, laid out [partition, chunk]
            t_idx = np.arange(rows).reshape(cj, CHUNK).T  # [128, cj]
            mask[:, col : col + cj] = np.where(t_idx < L, 0.0, MASK_NEG)
            col += cj
        in_maps.append(
            {"khat": khat, "vhat": vhat, "qrow": qrow, "maskv": mask}
        )
    return in_maps


def _build_program(cjs):
    """Trace the uniform SPMD Bass/Tile program for slot chunk counts cjs."""
    from contextlib import ExitStack

    import concourse.bass as bass
    import concourse.mybir as mybir
    from concourse import bacc, bass_isa, tile

    f32 = mybir.dt.float32
    bf16 = mybir.dt.bfloat16
    totc = sum(cjs)

    nc = bacc.Bacc("TRN2", target_bir_lowering=False, debug=False)
    kin = nc.dram_tensor("khat", [CHUNK, totc, D], f32, kind="ExternalInput").ap()
    vin = nc.dram_tensor("vhat", [CHUNK, totc, D], bf16, kind="ExternalInput").ap()
    qin = nc.dram_tensor(
        "qrow", [1, SLOTS_PER_CORE * D], f32, kind="ExternalInput"
    ).ap()
    min_ = nc.dram_tensor("maskv", [CHUNK, totc], f32, kind="ExternalInput").ap()
    out = nc.dram_tensor(
        "out", [SLOTS_PER_CORE, D], f32, kind="ExternalOutput"
    ).ap()

    with ExitStack() as ctx:
        tc = ctx.enter_context(tile.TileContext(nc))
        kpool = ctx.enter_context(tc.tile_pool(name="kpool", bufs=4))
        vpool = ctx.enter_context(tc.tile_pool(name="vpool", bufs=4))
        cpool = ctx.enter_context(tc.tile_pool(name="cpool", bufs=1))
        epool = ctx.enter_context(tc.tile_pool(name="epool", bufs=3))
        spool = ctx.enter_context(tc.tile_pool(name="spool", bufs=3))
        pcpool = ctx.enter_context(tc.tile_pool(name="pcpool", bufs=4, space="PSUM"))

        # ---- constants ----
        qsb = cpool.tile([1, SLOTS_PER_CORE * D], f32, tag="qsb")
        nc.scalar.dma_start(qsb[:], qin)
        masks = cpool.tile([CHUNK, totc], f32, tag="masks")
        nc.scalar.dma_start(masks[:], min_)
        # scalar_tensor_tensor requires a full-shape `out`, but a [128,1]
        # tile broadcast over the free dim keeps the DVE write-port cost
        # (and SBUF footprint) minimal -- only accum_out is consumed.
        dummy = cpool.tile([CHUNK, 1], f32, tag="stt_dummy")

        # ---- replicate each slot's query to all 128 partitions ----
        qreps = []
        for j in range(SLOTS_PER_CORE):
            qr = cpool.tile([CHUNK, D], f32, tag=f"qrep{j}")
            nc.gpsimd.partition_broadcast(qr[:], qsb[0:1, j * D : (j + 1) * D])
            qreps.append(qr)

        col = 0
        for j, cj in enumerate(cjs):
            # ---------- energy phase ----------
            # Sub-tile split: a small first transfer on the first slot lets
            # the DVE start within ~1.5us instead of waiting for a full 2 MB.
            if j == 0:
                splits = [1, 3, 4] + [SUB] * 8
            else:
                splits = [SUB] * 16
            etile = epool.tile([CHUNK, cj], f32, tag="E")
            s0 = 0
            for ns in splits:
                if s0 >= cj:
                    break
                ns = min(ns, cj - s0)
                ktile = kpool.tile([CHUNK, ns * D], f32, tag="kt")
                src = kin[:, col + s0 : col + s0 + ns, :]
                nc.sync.dma_start(ktile[:], src)
                for c in range(ns):
                    cs = s0 + c
                    nc.vector.scalar_tensor_tensor(
                        out=dummy.broadcast_to((CHUNK, D)),
                        in0=ktile[:, c * D : (c + 1) * D],
                        scalar=1.0,
                        in1=qreps[j][:],
                        op0=mybir.AluOpType.mult,
                        op1=mybir.AluOpType.mult,
                        accum_out=etile[:, cs : cs + 1],
                    )
                s0 += ns
            # apply the -1e6 padding mask
            nc.vector.tensor_add(etile[:], etile[:], masks[:, col : col + cj])

            # ---------- softmax ----------
            mx = spool.tile([CHUNK, 1], f32, tag="mx")
            nc.vector.reduce_max(mx[:], etile[:], axis=mybir.AxisListType.X)
            mall = spool.tile([CHUNK, 1], f32, tag="mall")
            nc.gpsimd.partition_all_reduce(
                mall[:], mx[:], CHUNK, bass_isa.ReduceOp.max
            )
            bias = spool.tile([CHUNK, 1], f32, tag="bias")
            nc.vector.tensor_scalar_mul(bias[:], mall[:], -1.0)
            atile = epool.tile([CHUNK, cj], bf16, tag="A")
            spart = spool.tile([CHUNK, 1], f32, tag="spart")
            nc.scalar.activation(
                atile[:],
                etile[:],
                mybir.ActivationFunctionType.Exp,
                bias=bias[:],
                scale=1.0,
                accum_out=spart[:],
            )
            sall = spool.tile([CHUNK, 1], f32, tag="sall")
            nc.gpsimd.partition_all_reduce(
                sall[:], spart[:], CHUNK, bass_isa.ReduceOp.add
            )
            rinv = spool.tile([1, 1], f32, tag="rinv")
            nc.vector.reciprocal(rinv[:], sall[0:1, 0:1])

            # ---------- context phase ----------
            pctx = pcpool.tile([1, D], f32, tag="pc")
            for s0 in range(0, cj, SUB):
                ns = min(SUB, cj - s0)
                vtile = vpool.tile([CHUNK, ns * D], bf16, tag="vt")
                src = vin[:, col + s0 : col + s0 + ns, :]
                nc.scalar.dma_start(vtile[:], src)
                for c in range(ns):
                    cs = s0 + c
                    nc.tensor.matmul(
                        pctx[:],
                        atile[:, cs : cs + 1],
                        vtile[:, c * D : (c + 1) * D],
                        start=(cs == 0),
                        stop=(cs == cj - 1),
                    )
            ob = spool.tile([1, D], f32, tag="ob")
            nc.scalar.mul(ob[:], pctx[:], rinv[:])
            nc.gpsimd.dma_start(out[j : j + 1, :], ob[:])

            col += cj

    nc.compile()
    return nc


def _get_program(cjs):
    if cjs not in _PROGRAM_CACHE:
        _PROGRAM_CACHE[cjs] = _build_program(cjs)
    return _PROGRAM_CACHE[cjs]


def run(query, key, value, lens, trace=False):
    """Run on 8 cores; returns (output [64, 512] fp32, BassKernelResults)."""
    from concourse.bass_utils import run_bass_kernel_spmd

    lens_arr = np.asarray(lens).astype(np.int64)
    zero_lens = lens_arr == 0
    leff, cjs, assign = _plan(lens_arr)
    nc = _get_program(cjs)
    in_maps = _pack_inputs(query, key, value, leff, zero_lens, cjs, assign)
    res = run_bass_kernel_spmd(
        nc, in_maps, list(range(NCORES)), trace=trace
    )
    out_full = np.empty((N, D), dtype=np.float32)
    for i in range(NCORES):
        ocore = res.results[i]["out"]
        for j in range(SLOTS_PER_CORE):
            out_full[assign[i][j]] = ocore[j]
    return out_full, res


def kernel(query, key, value, lens):
    out, _ = run(query, key, value, lens, trace=False)
    return out


# revision 22
# speedup vs baseline: 1.2459x; 1.0460x over previous
"""Masked single-query attention (N=64, T=2048, D=512) on 8 Trainium2 cores.

Reference computation per batch element n:
    energy[t] = sum_d key[t, n, :] . query[n, :]        (t < lens[n], else -1e9)
    attn      = softmax(energy)
    out[n]    = sum_t attn[t] * value[t, n, :]

Strategy:
  * Data-parallel over batch: each core handles 8 batch elements.
  * The mask means rows t >= lens[n] of key/value never contribute, so the
    host packs only the first lens[n] rows of K/V per element (rounded up to
    a 128-row chunk granularity) into per-core contiguous buffers, cutting
    HBM traffic roughly in half (memory-bound kernel).
  * All 8 cores must run the SAME program (SPMD), so the 64 batch elements
    are sorted by effective length and dealt round-robin into 8 "slots";
    slot j processes ceil(max_len_in_group / 128) chunks on every core.
    Rows past an element's own length are zero-padded; a per-(slot, chunk)
    additive mask vector (0 or -1e6) fed to the reduce's init-value operand
    reproduces the -1e9 masking exactly after softmax.
  * On-device per slot: energy via DVE tensor_tensor_reduce against the
    replicated query (keeps K in its natural [t, d] layout -- no transpose,
    and the 4x fp32 matmul penalty is avoided for the large contraction);
    softmax (max via PE transpose, exp+row-sum fused on ACT); context via
    PE matmuls attn_chunk.T @ V_chunk accumulated in PSUM.
  * lens[n] == 0 reproduces the reference exactly: softmax of an all-masked
    row is uniform over all T positions, so such elements are packed as
    2048 zero K rows (energy 0) with the full V.
"""

import sys

if "/opt/trn_rl_repo" not in sys.path:
    sys.path.insert(0, "/opt/trn_rl_repo")

import numpy as np

N, T, D = 64, 2048, 512
NCORES = 8
SLOTS_PER_CORE = N // NCORES
CHUNK = 128          # t-rows per energy/context chunk (partition dim)
SUB = 8              # chunks per DMA transfer ([128, SUB*512] tiles)
MASK_NEG = -1.0e6    # additive energy mask for padded rows

_PROGRAM_CACHE = {}


def _plan(lens):
    """Sort batch elements by effective length, deal into 8 slots x 8 cores.

    Returns (leff, cjs, assign) where assign[i][j] is the global batch index
    handled by core i in slot j, and cjs[j] is that slot's chunk count
    (shared by all cores so the SPMD program is uniform).
    """
    lens = np.asarray(lens).astype(np.int64)
    leff = np.where(lens == 0, T, lens)
    order = np.argsort(-leff, kind="stable")
    cjs = []
    assign = [[None] * SLOTS_PER_CORE for _ in range(NCORES)]
    for j in range(SLOTS_PER_CORE):
        grp = order[j * NCORES : (j + 1) * NCORES]
        cj = int(-(-int(leff[grp].max()) // CHUNK))  # ceil
        cjs.append(max(cj, 1))
        for i in range(NCORES):
            assign[i][j] = int(grp[i])
    return leff, tuple(cjs), assign


def _pack_inputs(query, key, value, leff, zero_lens, cjs, assign):
    """Build the per-core DRAM images: packed K/V, query row, energy mask.

    V is packed as bf16: the context contraction runs on the PE in bf16
    (fp32 matmuls cost 4 cycles/row vs 1 for bf16) and softmax weights are
    in [0, 1], so bf16 V costs ~2^-9 relative error while cutting the
    V half of HBM traffic in two (this kernel is HBM-bound).
    """
    import ml_dtypes

    query = np.ascontiguousarray(np.asarray(query, dtype=np.float32))
    key = np.asarray(key, dtype=np.float32)
    value = np.asarray(value, dtype=np.float32)
    totc = sum(cjs)
    in_maps = []
    for i in range(NCORES):
        # Partition-major packing [128, chunk, 512]: each SBUF partition's
        # DMA read is chunk-contiguous (8 KB runs for fp32 K at SUB=8)
        # instead of 2 KB strided rows -- bigger descriptors, better HBM rate.
        khat = np.zeros((CHUNK, totc, D), dtype=np.float16)
        vhat = np.zeros((CHUNK, totc, D), dtype=ml_dtypes.bfloat16)
        mask = np.zeros((CHUNK, totc), dtype=np.float32)
        qrow = np.zeros((1, SLOTS_PER_CORE * D), dtype=np.float16)
        col = 0
        for j, cj in enumerate(cjs):
            n = assign[i][j]
            L = int(leff[n])
            rows = cj * CHUNK
            # K rows: real rows for t < L unless the element is fully masked
            # (lens == 0 -> leff == T but energies must be 0, matching the
            # reference's uniform softmax over an all-masked row).
            kslot = np.zeros((rows, D), dtype=np.float16)
            if not zero_lens[n]:
                kslot[:L] = key[:L, n, :]
            khat[:, col : col + cj, :] = kslot.reshape(cj, CHUNK, D).transpose(
                1, 0, 2
            )
            vslot = np.zeros((rows, D), dtype=np.float32)
            vslot[:L] = value[:L, n, :]
            vhat[:, col : col + cj, :] = (
                vslot.astype(ml_dtypes.bfloat16)
                .reshape(cj, CHUNK, D)
                .transpose(1, 0, 2)
            )
            qrow[0, j * D : (j + 1) * D] = query[n]
            # mask: 0 where t < L else MASK_NEG, laid out [partition, chunk]
            t_idx = np.arange(rows).reshape(cj, CHUNK).T  # [128, cj]
            mask[:, col : col + cj] = np.where(t_idx < L, 0.0, MASK_NEG)
            col += cj
        in_maps.append(
            {"khat": khat, "vhat": vhat, "qrow": qrow, "maskv": mask}
        )
    return in_maps


def _build_program(cjs):
    """Trace the uniform SPMD Bass/Tile program for slot chunk counts cjs."""
    from contextlib import ExitStack

    import concourse.bass as bass
    import concourse.mybir as mybir
    from concourse import bacc, bass_isa, tile

    f32 = mybir.dt.float32
    bf16 = mybir.dt.bfloat16
    f16 = mybir.dt.float16
    totc = sum(cjs)

    nc = bacc.Bacc("TRN2", target_bir_lowering=False, debug=False)
    kin = nc.dram_tensor("khat", [CHUNK, totc, D], f16, kind="ExternalInput").ap()
    vin = nc.dram_tensor("vhat", [CHUNK, totc, D], bf16, kind="ExternalInput").ap()
    qin = nc.dram_tensor(
        "qrow", [1, SLOTS_PER_CORE * D], f16, kind="ExternalInput"
    ).ap()
    min_ = nc.dram_tensor("maskv", [CHUNK, totc], f32, kind="ExternalInput").ap()
    out = nc.dram_tensor(
        "out", [SLOTS_PER_CORE, D], f32, kind="ExternalOutput"
    ).ap()

    with ExitStack() as ctx:
        tc = ctx.enter_context(tile.TileContext(nc))
        kpool = ctx.enter_context(tc.tile_pool(name="kpool", bufs=4))
        vpool = ctx.enter_context(tc.tile_pool(name="vpool", bufs=4))
        cpool = ctx.enter_context(tc.tile_pool(name="cpool", bufs=1))
        epool = ctx.enter_context(tc.tile_pool(name="epool", bufs=3))
        spool = ctx.enter_context(tc.tile_pool(name="spool", bufs=3))
        pcpool = ctx.enter_context(tc.tile_pool(name="pcpool", bufs=4, space="PSUM"))

        # ---- constants ----
        qsb = cpool.tile([1, SLOTS_PER_CORE * D], f16, tag="qsb")
        nc.scalar.dma_start(qsb[:], qin)
        masks = cpool.tile([CHUNK, totc], f32, tag="masks")
        nc.scalar.dma_start(masks[:], min_)
        # scalar_tensor_tensor requires a full-shape `out`, but a [128,1]
        # tile broadcast over the free dim keeps the DVE write-port cost
        # (and SBUF footprint) minimal -- only accum_out is consumed.
        dummy = cpool.tile([CHUNK, 1], f32, tag="stt_dummy")

        # ---- replicate each slot's query to all 128 partitions ----
        qreps = []
        for j in range(SLOTS_PER_CORE):
            qr = cpool.tile([CHUNK, D], f16, tag=f"qrep{j}")
            nc.gpsimd.partition_broadcast(qr[:], qsb[0:1, j * D : (j + 1) * D])
            qreps.append(qr)

        col = 0
        for j, cj in enumerate(cjs):
            # ---------- energy phase ----------
            # Sub-tile split: a small first transfer on the first slot lets
            # the DVE start within ~1.5us instead of waiting for a full 2 MB.
            if j == 0:
                splits = [1, 3, 4] + [SUB] * 8
            else:
                splits = [SUB] * 16
            etile = epool.tile([CHUNK, cj], f32, tag="E")
            s0 = 0
            for ns in splits:
                if s0 >= cj:
                    break
                ns = min(ns, cj - s0)
                ktile = kpool.tile([CHUNK, ns * D], f16, tag="kt")
                src = kin[:, col + s0 : col + s0 + ns, :]
                nc.sync.dma_start(ktile[:], src)
                for c in range(ns):
                    cs = s0 + c
                    nc.vector.scalar_tensor_tensor(
                        out=dummy.broadcast_to((CHUNK, D)),
                        in0=ktile[:, c * D : (c + 1) * D],
                        scalar=1.0,
                        in1=qreps[j][:],
                        op0=mybir.AluOpType.mult,
                        op1=mybir.AluOpType.mult,
                        accum_out=etile[:, cs : cs + 1],
                    )
                s0 += ns
            # apply the -1e6 padding mask
            nc.vector.tensor_add(etile[:], etile[:], masks[:, col : col + cj])

            # ---------- softmax ----------
            mx = spool.tile([CHUNK, 1], f32, tag="mx")
            nc.vector.reduce_max(mx[:], etile[:], axis=mybir.AxisListType.X)
            mall = spool.tile([CHUNK, 1], f32, tag="mall")
            nc.gpsimd.partition_all_reduce(
                mall[:], mx[:], CHUNK, bass_isa.ReduceOp.max
            )
            bias = spool.tile([CHUNK, 1], f32, tag="bias")
            nc.vector.tensor_scalar_mul(bias[:], mall[:], -1.0)
            atile = epool.tile([CHUNK, cj], bf16, tag="A")
            spart = spool.tile([CHUNK, 1], f32, tag="spart")
            nc.scalar.activation(
                atile[:],
                etile[:],
                mybir.ActivationFunctionType.Exp,
                bias=bias[:],
                scale=1.0,
                accum_out=spart[:],
            )
            sall = spool.tile([CHUNK, 1], f32, tag="sall")
            nc.gpsimd.partition_all_reduce(
                sall[:], spart[:], CHUNK, bass_isa.ReduceOp.add
            )
            rinv = spool.tile([1, 1], f32, tag="rinv")
            nc.vector.reciprocal(rinv[:], sall[0:1, 0:1])

            # ---------- context phase ----------
            pctx = pcpool.tile([1, D], f32, tag="pc")
            for s0 in range(0, cj, SUB):
                ns = min(SUB, cj - s0)
                vtile = vpool.tile([CHUNK, ns * D], bf16, tag="vt")
                src = vin[:, col + s0 : col + s0 + ns, :]
                nc.scalar.dma_start(vtile[:], src)
                for c in range(ns):
                    cs = s0 + c
                    nc.tensor.matmul(
                        pctx[:],
                        atile[:, cs : cs + 1],
                        vtile[:, c * D : (c + 1) * D],
                        start=(cs == 0),
                        stop=(cs == cj - 1),
                    )
            ob = spool.tile([1, D], f32, tag="ob")
            nc.scalar.mul(ob[:], pctx[:], rinv[:])
            nc.gpsimd.dma_start(out[j : j + 1, :], ob[:])

            col += cj

    nc.compile()
    return nc


def _get_program(cjs):
    if cjs not in _PROGRAM_CACHE:
        _PROGRAM_CACHE[cjs] = _build_program(cjs)
    return _PROGRAM_CACHE[cjs]


def run(query, key, value, lens, trace=False):
    """Run on 8 cores; returns (output [64, 512] fp32, BassKernelResults)."""
    from concourse.bass_utils import run_bass_kernel_spmd

    lens_arr = np.asarray(lens).astype(np.int64)
    zero_lens = lens_arr == 0
    leff, cjs, assign = _plan(lens_arr)
    nc = _get_program(cjs)
    in_maps = _pack_inputs(query, key, value, leff, zero_lens, cjs, assign)
    res = run_bass_kernel_spmd(
        nc, in_maps, list(range(NCORES)), trace=trace
    )
    out_full = np.empty((N, D), dtype=np.float32)
    for i in range(NCORES):
        ocore = res.results[i]["out"]
        for j in range(SLOTS_PER_CORE):
            out_full[assign[i][j]] = ocore[j]
    return out_full, res


def kernel(query, key, value, lens):
    out, _ = run(query, key, value, lens, trace=False)
    return out
